# revision 24
# baseline (speedup 1.0000x reference)
"""NeRF renderer Bass kernel for TRN2 (8-core data-parallel over rays).

Restructured math (validated vs reference to ~5e-6 rel in numpy):
  pre[h, (n,s)] = A[ray, h] + t[s] * B[ray, h]        (A/B precomputed on host)
  h = relu(pre);  sigma = softplus(h @ w_sigma);  rgb = sigmoid(h @ W_color)
  weights via exact exclusive-cumprod (hardware tensor_tensor_scan).

Per-core geometry (2048 rays, 128 samples, hidden 64):
  ray-local index u = 128*j + p   (j = ray-tile 0..15, p = partition 0..127)
  within tile j:  p = 64*mh + 16*Q + 8*e + 4*hf + n
  group g = 8*mh + 2*Q + hf  (8 rays: even chunk e=0 n=0..3, odd chunk e=1)

Device pipeline per core:
  per ray-tile j: DMA block-diag ABi [16, 2048] (f32r);
    per O8-half mh: 4x { mm1 x2 (K=16, M=128, N=512, f32r) -> H2 psum
                         relu (ACT/DVE) -> h2s sbuf (f32r)
                         mm2 x2 (K=128, M=128 accum over quads) -> O8 psum }
    drain O8 -> sbuf -> dump valid rows to DRAM -> gather sigma/rgb
      into ray-major [128, J*S] tiles
  compositing: softplus (exp/ln), exp, tensor_tensor_scan cumprod,
    per-block reductions, sigmoid-via-tanh, dense output DMAs.
"""
import dataclasses
import os
import numpy as np
from contextlib import ExitStack

import concourse.bass as bass
import concourse.tile as tile
from concourse import bacc, mybir

f32 = np.float32
dt = mybir.dt
AF = mybir.ActivationFunctionType
ALU = mybir.AluOpType

NCORES = 8
R = 2048          # rays per core
J = 16            # ray tiles per core
S = 128           # samples per ray
H = 64            # hidden dim
ZCLAMP = 4.0
MIN_NEAR = 0.05
USE_F32R = True
MMDT = dt.float32r if USE_F32R else dt.float32
NCORES_ENV = int(os.environ.get("NERF_CORES", "8"))


def rep(ap, offset_delta, new_ap):
    return dataclasses.replace(ap, offset=ap.offset + offset_delta, ap=new_ap)


# ---------------------------------------------------------------- host side
def build_consts(W1, W2, b1, w_sigma, W_color):
    t = np.linspace(0.0, 1.0, S, dtype=f32)
    # mm2 weights: M=128, one [128,128] block per quad Q (values at cols 32Q+r)
    # col order r = 2*v + e  (v: 0=sigma,1=r,2=g,3=b ; e: chunk parity)
    lhsT2 = np.zeros((128, 4, 128), f32)
    wout = np.concatenate([w_sigma, W_color], 1).astype(f32)      # [64, 4]
    for Q in range(4):
        for v in range(4):
            lhsT2[0:64, Q, 8 * Q + 2 * v + 0] = wout[:, v]
            lhsT2[64:128, Q, 8 * Q + 2 * v + 1] = wout[:, v]
    lhsT2 = lhsT2.reshape(128, 4 * 128)
    # mm1 rhs constant [16, 512]: cols (n:4, s:128)
    RC = np.zeros((16, 512), f32)
    for n in range(4):
        RC[n, n * S:(n + 1) * S] = 1.0            # A-selector
        RC[4 + n, n * S:(n + 1) * S] = t          # B-selector * t
    RC[8:16] = RC[0:8]
    t_full = np.tile(t[None, :], (128, J)).astype(f32)            # [128, 2048]
    d1 = np.zeros((128, J * S), f32)
    d1[:, ::S] = 1.0                                              # scan reset
    return dict(lhsT2=lhsT2, RC=RC, t_full=t_full, d1=d1)


def per_ray_host(o, d, W1, W2, b1):
    """o, d: [R, 3] one core. Returns ABi [J, 16, 2048], near2/rangez/negdelta
    in the [128, J] shuffled layout."""
    inv = (f32(1.0) / (d + f32(1e-15))).astype(f32)
    tmin = ((f32(-1.0) - o) * inv).astype(f32)
    tmax = ((f32(1.0) - o) * inv).astype(f32)
    near = np.minimum(tmin, tmax).max(-1)
    far = np.maximum(tmin, tmax).min(-1)
    miss = far < near
    near = np.where(miss, f32(1e9), near).astype(f32)
    far = np.where(miss, f32(1e9), far).astype(f32)
    near = np.maximum(near, f32(MIN_NEAR))
    rangez = (far - near).astype(f32)
    negdelta = (rangez * f32(-1.0 / (S - 1))).astype(f32)
    nearc = np.minimum(near, f32(ZCLAMP))
    farc = np.minimum(far, f32(ZCLAMP))
    rangec = (farc - nearc).astype(f32)
    P = (o + d * nearc[:, None]).astype(f32)
    Qv = (d * rangec[:, None]).astype(f32)
    A = (P @ W1 + d @ W2 + b1).astype(f32)        # [R, 64]
    Bq = (Qv @ W1).astype(f32)                    # [R, 64]

    # block-diag ABi: [J, 16, 16, 128]: (j, k, g, m128)
    # ray u = 128*j + p ; p = 64mh + 16Q + 8e + 4hf + n ; g = 8mh + 2Q + hf
    # K-rows: k = 8e + n (A), 8e + 4 + n (B); cols 64e + (0..63)
    pvec = np.arange(128)
    mh, Qq, e, hf, n = (pvec // 64, (pvec // 16) % 4, (pvec // 8) % 2,
                        (pvec // 4) % 2, pvec % 4)
    gv = 8 * mh + 2 * Qq + hf
    kA = 8 * e + n
    kB = 8 * e + 4 + n
    col0 = 64 * e
    A3 = A.reshape(J, 128, H)
    B3 = Bq.reshape(J, 128, H)
    abi = np.zeros((J, 16, 16, 128), f32)
    mgrid = np.arange(64)
    abi[:, kA[:, None], gv[:, None], (col0[:, None] + mgrid[None, :])] = A3
    abi[:, kB[:, None], gv[:, None], (col0[:, None] + mgrid[None, :])] = B3
    abi = np.ascontiguousarray(abi.reshape(J, 16, 16 * 128))

    def shuf(x):
        return np.ascontiguousarray(x.reshape(J, 128).T)
    return abi, shuf(near), shuf(rangez), shuf(negdelta)


def unshuffle_w(w_out):
    return w_out.reshape(128, J, S).transpose(1, 0, 2).reshape(R, S)


def unshuffle_scalar(x_out):
    return x_out.reshape(128, J).T.reshape(R)


def unshuffle_img(img_out):
    return img_out.reshape(128, J, 3).transpose(1, 0, 2).reshape(R, 3)


# ---------------------------------------------------------------- kernel body
def nerf_kernel(ctx: ExitStack, tc: tile.TileContext, outs, ins):
    nc = tc.nc
    (abi_in, near2_in, rangez_in, negdelta_in,
     lhsT2_in, RC_in, t_full_in, d1_in) = ins
    (w_out, depth_out, wsum_out, img_out) = outs
    V = nc.vector
    SC = nc.scalar

    const = ctx.enter_context(tc.tile_pool(name="const", bufs=1))
    pr = ctx.enter_context(tc.tile_pool(name="pr", bufs=1))
    work = ctx.enter_context(tc.tile_pool(name="work", bufs=1))
    abipool = ctx.enter_context(tc.tile_pool(name="abip", bufs=4))
    hpool = ctx.enter_context(tc.tile_pool(name="hp", bufs=4))
    ospool = ctx.enter_context(tc.tile_pool(name="osp", bufs=4))
    psum = ctx.enter_context(tc.tile_pool(name="psum", bufs=1, space="PSUM"))
    dram = ctx.enter_context(tc.tile_pool(name="dram", bufs=1, space="DRAM"))

    def cload(shape, src, dtype=dt.float32):
        tl = const.tile(shape, dtype, name=f"c_{src.tensor.name}")
        eng = nc.sync if src.dtype == dtype else nc.gpsimd
        eng.dma_start(tl[:], src[:])
        return tl

    lhsT2 = cload([128, 4 * 128], lhsT2_in, MMDT)
    RC = cload([16, 512], RC_in, MMDT)
    t_full = cload([128, J * S], t_full_in)
    d1c = cload([128, J * S], d1_in)
    near2 = cload([128, J], near2_in)
    rangez = cload([128, J], rangez_in)
    negdelta = cload([128, J], negdelta_in)

    sig_all = work.tile([128, J * S], dt.float32)
    rgb_all = [work.tile([128, J * S], dt.float32, name=f"rgb_all{c}",
                         tag=f"rgb{c}") for c in range(3)]
    o8_scr = dram.tile([J * 2, 32 * 1024], dt.float32)

    # ================= main loop
    half_done = [False]

    def main_tile(j):
        abi = abipool.tile([16, 16 * 128], MMDT, name="abi", tag="abi")
        nc.gpsimd.dma_start(abi[:], abi_in[j, :, :])
        for mh in range(2):
            o8 = psum.tile([128, 1024], dt.float32, name="o8", tag="o8", bufs=2)
            for Q in range(4):
                h2 = psum.tile([128, 1024], dt.float32, name="h2", tag="h2", bufs=2)
                for hf in range(2):
                    g = 8 * mh + 2 * Q + hf
                    nc.tensor.matmul(h2[:, 512 * hf:512 * (hf + 1)],
                                     abi[:, 128 * g:128 * (g + 1)],
                                     RC[:], start=True, stop=True)
                h2s = hpool.tile([128, 1024], MMDT, name="h2s", tag="h2s")
                if (8 * j + 4 * mh + Q) % 8 in (2, 5, 7):
                    V.tensor_scalar(h2s[:], h2[:], 0.0, None, ALU.max)
                else:
                    SC.activation(h2s[:], h2[:], AF.Relu)
                for hf in range(2):
                    nc.tensor.matmul(o8[:, 512 * hf:512 * (hf + 1)],
                                     lhsT2[:, 128 * Q:128 * (Q + 1)],
                                     h2s[:, 512 * hf:512 * (hf + 1)],
                                     start=(Q == 0), stop=(Q == 3))
            # drain O8 valid rows 0:32 -> sbuf
            o8s = ospool.tile([32, 1024], dt.float32, name="o8s", tag="o8s")
            if (2 * j + mh) % 2 == 0:
                SC.copy(o8s[:], o8[0:32, :])
            else:
                V.tensor_copy(o8s[:], o8[0:32, :])
            # single dump [32, 1024] -> scratch slot (contiguous)
            slot = o8_scr[2 * j + mh, :]
            dmp = nc.sync if mh == 0 else nc.gpsimd
            dmp.dma_start(rep(slot, 0, [[1024, 32], [1, 1024]]), o8s[:])
        # gathers (both mh at once): per v: dst [128, 128] s-block;
        # src addr = mh*32768 + (8Q + 2v + e)*1024 + hf*512 + n*128 + s
        slot0 = o8_scr[2 * j, :]
        for v in range(4):
            tgt = sig_all if v == 0 else rgb_all[v - 1]
            dst = tgt[:, S * j:S * (j + 1)]
            src = rep(slot0, 2 * v * 1024,
                      [[32768, 2], [8192, 4], [1, 2048]])
            nc.sync.dma_start(dst, src)

    # ================= compositing (j-half granularity, overlapped)
    # persistent across halves
    w_all = work.tile([128, J * S], dt.float32)
    sd = work.tile([128, J * S], dt.float32)
    nc.vector.memset(sd[:], 0.0)
    wsum = pr.tile([128, J], dt.float32)
    wtsum = pr.tile([128, J], dt.float32)

    def composite_sigma(j0, j1):
        c0, c1 = S * j0, S * j1
        jn = j1 - j0
        sig_e = work.tile([128, jn * S], dt.float32, tag="bigB", name=f"sig_e{j0}")
        SC.activation(sig_e[:], sig_all[:, c0:c1], AF.Exp)
        sig_e1 = work.tile([128, jn * S], dt.float32, tag="bigC", name=f"sig_e1{j0}")
        V.tensor_scalar(sig_e1[:], sig_e[:], 1.0, None, ALU.add)
        sig_sp = work.tile([128, jn * S], dt.float32, tag="bigA", name=f"sig_sp{j0}")
        SC.activation(sig_sp[:], sig_e1[:], AF.Ln)                 # softplus
        negds = work.tile([128, jn * S], dt.float32, tag="bigB", name=f"negds{j0}")
        for j in range(j0, j1):
            V.tensor_scalar(negds[:, S * (j - j0):S * (j - j0 + 1)],
                            sig_sp[:, S * (j - j0):S * (j - j0 + 1)],
                            negdelta[:, j:j + 1], None, ALU.mult)
        am1 = work.tile([128, jn * S], dt.float32, tag="bigC", name=f"am1{j0}")
        SC.activation(am1[:], negds[:], AF.Exp)                    # 1 - alpha
        sh = work.tile([128, jn * S], dt.float32, tag="bigA", name=f"sh{j0}")
        V.tensor_scalar(sh[:], am1[:], 1e-15, None, ALU.add)
        sd_v = sd[:, c0:c1].rearrange("p (j s) -> p j s", s=S)
        sh_v = sh[:].rearrange("p (j s) -> p j s", s=S)
        nc.gpsimd.tensor_copy(sd_v[:, :, 1:S], sh_v[:, :, 0:S - 1])
        tex = work.tile([128, jn * S], dt.float32, tag="bigB", name=f"tex{j0}")
        V.tensor_tensor_scan(tex[:], sd[:, c0:c1], d1c[:, c0:c1], 0.0,
                             ALU.mult, ALU.add)
        alpha = work.tile([128, jn * S], dt.float32, tag="bigA", name=f"alpha{j0}")
        V.tensor_scalar(alpha[:], am1[:], -1.0, 1.0, ALU.mult, ALU.add)
        V.tensor_tensor(w_all[:, c0:c1], alpha[:], tex[:], ALU.mult)
        nc.scalar.dma_start(w_out[:, c0:c1], w_all[:, c0:c1])
        w_v = w_all[:, c0:c1].rearrange("p (j s) -> p j s", s=S)
        V.tensor_reduce(wsum[:, j0:j1], w_v, mybir.AxisListType.X, ALU.add)
        wt = work.tile([128, jn * S], dt.float32, tag="bigC", name=f"wt{j0}")
        nc.gpsimd.tensor_tensor(wt[:], w_all[:, c0:c1], t_full[:, c0:c1], ALU.mult)
        V.tensor_reduce(wtsum[:, j0:j1], wt[:].rearrange("p (j s) -> p j s", s=S),
                        mybir.AxisListType.X, ALU.add)

    # image part, deferred so tanh needs one table load at the very end
    def composite_image():
        nc.scalar.dma_start(wsum_out[:], wsum[:])
        dep1 = pr.tile([128, J], dt.float32)
        V.tensor_tensor(dep1[:], near2[:], wsum[:], ALU.mult)
        dep2 = pr.tile([128, J], dt.float32)
        V.tensor_tensor(dep2[:], rangez[:], wtsum[:], ALU.mult)
        depth = pr.tile([128, J], dt.float32)
        V.tensor_tensor(depth[:], dep1[:], dep2[:], ALU.add)
        nc.scalar.dma_start(depth_out[:], depth[:])
        img = pr.tile([128, 3 * J], dt.float32)
        imgv = img[:].rearrange("p (j c) -> p j c", c=3)
        for c in range(3):
            th = work.tile([128, J * S], dt.float32, name=f"th{c}", tag="bigB")
            SC.activation(th[:], rgb_all[c][:], AF.Tanh, scale=0.5)
            wth = work.tile([128, J * S], dt.float32, name=f"wth{c}", tag="bigC")
            eng = nc.gpsimd if c != 2 else V
            eng.tensor_tensor(wth[:], w_all[:], th[:], ALU.mult)
            sc_ = pr.tile([128, J], dt.float32, name=f"sc{c}", tag="scc")
            V.tensor_reduce(sc_[:], wth[:].rearrange("p (j s) -> p j s", s=S),
                            mybir.AxisListType.X, ALU.add)
            tmp2 = pr.tile([128, J], dt.float32, name=f"tmp2{c}", tag="tmp2")
            V.tensor_tensor(tmp2[:], wsum[:], sc_[:], ALU.add)
            V.tensor_scalar(imgv[:, :, c:c + 1].squeeze(-1), tmp2[:], 0.5, None,
                            ALU.mult)
        nc.scalar.dma_start(img_out[:], img[:])

    for j in range(J):
        main_tile(j)
        if (j + 1) % 4 == 0 and j < J - 1:
            composite_sigma(j - 3, j + 1)
    composite_sigma(J - 4, J)
    composite_image()


# ======================================================================
# SPMD entry
# ======================================================================
_CACHE = {}


def _build_bass():
    nc = bacc.Bacc("TRN2", target_bir_lowering=False, debug=False,
                   num_devices=NCORES)
    ins_specs = [
        ("abi_in", [J, 16, 16 * 128], MMDT),
        ("near2_in", [128, J], dt.float32),
        ("rangez_in", [128, J], dt.float32),
        ("negdelta_in", [128, J], dt.float32),
        ("lhsT2_in", [128, 4 * 128], MMDT),
        ("RC_in", [16, 512], MMDT),
        ("t_full_in", [128, J * S], dt.float32),
        ("d1_in", [128, J * S], dt.float32),
    ]
    outs_specs = [
        ("w_out", [128, J * S]), ("depth_out", [128, J]),
        ("wsum_out", [128, J]), ("img_out", [128, 3 * J]),
    ]
    ins = [nc.dram_tensor(n, s, dty, kind="ExternalInput").ap()
           for n, s, dty in ins_specs]
    outs = [nc.dram_tensor(n, s, dt.float32, kind="ExternalOutput").ap()
            for n, s in outs_specs]
    with tile.TileContext(nc) as tc:
        with ExitStack() as ctx:
            nerf_kernel(ctx, tc, outs, ins)
    nc.compile()
    return nc


def get_nc():
    if "nc" not in _CACHE:
        _CACHE["nc"] = _build_bass()
    return _CACHE["nc"]


def make_in_maps(rays_o, rays_d, W1, W2, b1, w_sigma, W_color):
    consts = build_consts(W1, W2, b1, w_sigma, W_color)
    o_flat = np.ascontiguousarray(rays_o.reshape(-1, 3), f32)
    d_flat = np.ascontiguousarray(rays_d.reshape(-1, 3), f32)
    in_maps = []
    for c in range(NCORES):
        abi, near2, rangez, negdelta = per_ray_host(
            o_flat[c * R:(c + 1) * R], d_flat[c * R:(c + 1) * R],
            W1.astype(f32), W2.astype(f32), b1.astype(f32))
        in_maps.append({
            "abi_in": abi, "near2_in": near2, "rangez_in": rangez,
            "negdelta_in": negdelta, "lhsT2_in": consts["lhsT2"],
            "RC_in": consts["RC"], "t_full_in": consts["t_full"],
            "d1_in": consts["d1"],
        })
    return in_maps


def assemble_outputs(results, B, N):
    imgs, depths, ws, wsums = [], [], [], []
    for c in range(len(results)):
        r = results[c]
        imgs.append(unshuffle_img(r["img_out"]))
        depths.append(unshuffle_scalar(r["depth_out"]))
        ws.append(unshuffle_w(r["w_out"]))
        wsums.append(unshuffle_scalar(r["wsum_out"]))
    image = np.concatenate(imgs, 0).reshape(B, N, 3)
    depth = np.concatenate(depths, 0).reshape(B, N)
    weights = np.concatenate(ws, 0).reshape(B, N, S)
    weights_sum = np.concatenate(wsums, 0).reshape(B, N)
    return image, depth, weights, weights_sum


def run_spmd(inputs, trace=False):
    from concourse import bass_utils
    nc = get_nc()
    in_maps = make_in_maps(**inputs)[:NCORES_ENV]
    res = bass_utils.run_bass_kernel_spmd(
        nc, in_maps, core_ids=list(range(NCORES_ENV)), trace=trace)
    return res


def kernel(rays_o, rays_d, W1, W2, b1, w_sigma, W_color):
    inputs = dict(rays_o=np.asarray(rays_o, f32), rays_d=np.asarray(rays_d, f32),
                  W1=np.asarray(W1, f32), W2=np.asarray(W2, f32),
                  b1=np.asarray(b1, f32), w_sigma=np.asarray(w_sigma, f32),
                  W_color=np.asarray(W_color, f32))
    B, N = inputs["rays_o"].shape[:2]
    res = run_spmd(inputs)
    return assemble_outputs(res.results, B, N)


# revision 28
# speedup vs baseline: 1.0145x; 1.0145x over previous
"""NeRF renderer Bass kernel for TRN2 (8-core data-parallel over rays).

Restructured math (validated vs reference to ~5e-6 rel in numpy):
  pre[h, (n,s)] = A[ray, h] + t[s] * B[ray, h]        (A/B precomputed on host)
  h = relu(pre);  sigma = softplus(h @ w_sigma);  rgb = sigmoid(h @ W_color)
  weights via exact exclusive-cumprod (hardware tensor_tensor_scan).

Per-core geometry (2048 rays, 128 samples, hidden 64):
  ray-local index u = 128*j + p   (j = ray-tile 0..15, p = partition 0..127)
  within tile j:  p = 64*mh + 16*Q + 8*e + 4*hf + n
  group g = 8*mh + 2*Q + hf  (8 rays: even chunk e=0 n=0..3, odd chunk e=1)

Device pipeline per core:
  per ray-tile j: DMA block-diag ABi [16, 2048] (f32r);
    per O8-half mh: 4x { mm1 x2 (K=16, M=128, N=512, f32r) -> H2 psum
                         relu (ACT/DVE) -> h2s sbuf (f32r)
                         mm2 x2 (K=128, M=128 accum over quads) -> O8 psum }
    drain O8 -> sbuf -> dump valid rows to DRAM -> gather sigma/rgb
      into ray-major [128, J*S] tiles
  compositing: softplus (exp/ln), exp, tensor_tensor_scan cumprod,
    per-block reductions, sigmoid-via-tanh, dense output DMAs.
"""
import dataclasses
import os
import numpy as np
from contextlib import ExitStack

import concourse.bass as bass
import concourse.tile as tile
from concourse import bacc, mybir

f32 = np.float32
dt = mybir.dt
AF = mybir.ActivationFunctionType
ALU = mybir.AluOpType

NCORES = 8
R = 2048          # rays per core
J = 16            # ray tiles per core
S = 128           # samples per ray
H = 64            # hidden dim
ZCLAMP = 4.0
MIN_NEAR = 0.05
USE_F32R = True
MMDT = dt.float32r if USE_F32R else dt.float32
NCORES_ENV = int(os.environ.get("NERF_CORES", "8"))


def rep(ap, offset_delta, new_ap):
    return dataclasses.replace(ap, offset=ap.offset + offset_delta, ap=new_ap)


# ---------------------------------------------------------------- host side
def build_consts(W1, W2, b1, w_sigma, W_color):
    t = np.linspace(0.0, 1.0, S, dtype=f32)
    # mm2 weights: M=128, one [128,128] block per quad Q (values at cols 32Q+r)
    # col order r = 2*v + e  (v: 0=sigma,1=r,2=g,3=b ; e: chunk parity)
    lhsT2 = np.zeros((128, 4, 128), f32)
    wout = np.concatenate([w_sigma, W_color], 1).astype(f32)      # [64, 4]
    for Q in range(4):
        for v in range(4):
            lhsT2[0:64, Q, 8 * Q + 2 * v + 0] = wout[:, v]
            lhsT2[64:128, Q, 8 * Q + 2 * v + 1] = wout[:, v]
    lhsT2 = lhsT2.reshape(128, 4 * 128)
    # mm1 rhs constant [16, 512]: cols (n:4, s:128)
    RC = np.zeros((16, 512), f32)
    for n in range(4):
        RC[n, n * S:(n + 1) * S] = 1.0            # A-selector
        RC[4 + n, n * S:(n + 1) * S] = t          # B-selector * t
    RC[8:16] = RC[0:8]
    t_full = np.tile(t[None, :], (128, J)).astype(f32)            # [128, 2048]
    d1 = np.zeros((128, J * S), f32)
    d1[:, ::S] = 1.0                                              # scan reset
    return dict(lhsT2=lhsT2, RC=RC, t_full=t_full, d1=d1)


def per_ray_host(o, d, W1, W2, b1):
    """o, d: [R, 3] one core. Returns ABi [J, 16, 2048], near2/rangez/negdelta
    in the [128, J] shuffled layout."""
    inv = (f32(1.0) / (d + f32(1e-15))).astype(f32)
    tmin = ((f32(-1.0) - o) * inv).astype(f32)
    tmax = ((f32(1.0) - o) * inv).astype(f32)
    near = np.minimum(tmin, tmax).max(-1)
    far = np.maximum(tmin, tmax).min(-1)
    miss = far < near
    near = np.where(miss, f32(1e9), near).astype(f32)
    far = np.where(miss, f32(1e9), far).astype(f32)
    near = np.maximum(near, f32(MIN_NEAR))
    rangez = (far - near).astype(f32)
    negdelta = (rangez * f32(-1.0 / (S - 1))).astype(f32)
    nearc = np.minimum(near, f32(ZCLAMP))
    farc = np.minimum(far, f32(ZCLAMP))
    rangec = (farc - nearc).astype(f32)
    P = (o + d * nearc[:, None]).astype(f32)
    Qv = (d * rangec[:, None]).astype(f32)
    A = (P @ W1 + d @ W2 + b1).astype(f32)        # [R, 64]
    Bq = (Qv @ W1).astype(f32)                    # [R, 64]

    # block-diag ABi: [J, 16, 16, 128]: (j, k, g, m128)
    # ray u = 128*j + p ; p = 64mh + 16Q + 8e + 4hf + n ; g = 8mh + 2Q + hf
    # K-rows: k = 8e + n (A), 8e + 4 + n (B); cols 64e + (0..63)
    pvec = np.arange(128)
    mh, Qq, e, hf, n = (pvec // 64, (pvec // 16) % 4, (pvec // 8) % 2,
                        (pvec // 4) % 2, pvec % 4)
    gv = 8 * mh + 2 * Qq + hf
    kA = 8 * e + n
    kB = 8 * e + 4 + n
    col0 = 64 * e
    A3 = A.reshape(J, 128, H)
    B3 = Bq.reshape(J, 128, H)
    abi = np.zeros((J, 16, 16, 128), f32)
    mgrid = np.arange(64)
    abi[:, kA[:, None], gv[:, None], (col0[:, None] + mgrid[None, :])] = A3
    abi[:, kB[:, None], gv[:, None], (col0[:, None] + mgrid[None, :])] = B3
    abi = np.ascontiguousarray(abi.reshape(J, 16, 16 * 128))

    def shuf(x):
        return np.ascontiguousarray(x.reshape(J, 128).T)
    return abi, shuf(near), shuf(rangez), shuf(negdelta)


def unshuffle_w(w_out):
    return w_out.reshape(128, J, S).transpose(1, 0, 2).reshape(R, S)


def unshuffle_scalar(x_out):
    return x_out.reshape(128, J).T.reshape(R)


def unshuffle_img(img_out):
    return img_out.reshape(128, J, 3).transpose(1, 0, 2).reshape(R, 3)


# ---------------------------------------------------------------- kernel body
def nerf_kernel(ctx: ExitStack, tc: tile.TileContext, outs, ins):
    nc = tc.nc
    (abi_in, near2_in, rangez_in, negdelta_in,
     lhsT2_in, RC_in, t_full_in, d1_in) = ins
    (w_out, depth_out, wsum_out, img_out) = outs
    V = nc.vector
    SC = nc.scalar

    const = ctx.enter_context(tc.tile_pool(name="const", bufs=1))
    pr = ctx.enter_context(tc.tile_pool(name="pr", bufs=1))
    work = ctx.enter_context(tc.tile_pool(name="work", bufs=1))
    abipool = ctx.enter_context(tc.tile_pool(name="abip", bufs=4))
    hpool = ctx.enter_context(tc.tile_pool(name="hp", bufs=4))
    ospool = ctx.enter_context(tc.tile_pool(name="osp", bufs=4))
    psum = ctx.enter_context(tc.tile_pool(name="psum", bufs=1, space="PSUM"))
    dram = ctx.enter_context(tc.tile_pool(name="dram", bufs=1, space="DRAM"))

    def cload(shape, src, dtype=dt.float32):
        tl = const.tile(shape, dtype, name=f"c_{src.tensor.name}")
        eng = nc.sync if src.dtype == dtype else nc.gpsimd
        eng.dma_start(tl[:], src[:])
        return tl

    lhsT2 = cload([128, 4 * 128], lhsT2_in, MMDT)
    RC = cload([16, 512], RC_in, MMDT)
    t_full = cload([128, J * S], t_full_in)
    d1c = cload([128, J * S], d1_in)
    near2 = cload([128, J], near2_in)
    rangez = cload([128, J], rangez_in)
    negdelta = cload([128, J], negdelta_in)

    sig_all = work.tile([128, J * S], dt.float32)
    rgb_all = [work.tile([128, J * S], dt.float32, name=f"rgb_all{c}",
                         tag=f"rgb{c}") for c in range(3)]
    o8_scr = dram.tile([J * 2, 32 * 1024], dt.float32)

    # ================= main loop
    half_done = [False]

    def main_tile(j):
        abi = abipool.tile([16, 16 * 128], MMDT, name="abi", tag="abi")
        nc.gpsimd.dma_start(abi[:], abi_in[j, :, :])
        for mh in range(2):
            o8 = psum.tile([128, 1024], dt.float32, name="o8", tag="o8", bufs=2)
            for Q in range(4):
                h2 = psum.tile([128, 1024], dt.float32, name="h2", tag="h2", bufs=2)
                for hf in range(2):
                    g = 8 * mh + 2 * Q + hf
                    nc.tensor.matmul(h2[:, 512 * hf:512 * (hf + 1)],
                                     abi[:, 128 * g:128 * (g + 1)],
                                     RC[:], start=True, stop=True)
                h2s = hpool.tile([128, 1024], MMDT, name="h2s", tag="h2s")
                if (8 * j + 4 * mh + Q) % 8 in (2, 5, 7):
                    V.tensor_scalar(h2s[:], h2[:], 0.0, None, ALU.max)
                else:
                    SC.activation(h2s[:], h2[:], AF.Relu)
                for hf in range(2):
                    nc.tensor.matmul(o8[:, 512 * hf:512 * (hf + 1)],
                                     lhsT2[:, 128 * Q:128 * (Q + 1)],
                                     h2s[:, 512 * hf:512 * (hf + 1)],
                                     start=(Q == 0), stop=(Q == 3))
            # drain O8 valid rows 0:32 -> sbuf
            o8s = ospool.tile([32, 1024], dt.float32, name="o8s", tag="o8s")
            if (2 * j + mh) % 2 == 0:
                SC.copy(o8s[:], o8[0:32, :])
            else:
                V.tensor_copy(o8s[:], o8[0:32, :])
            # single dump [32, 1024] -> scratch slot (contiguous)
            slot = o8_scr[2 * j + mh, :]
            dmp = nc.sync if mh == 0 else nc.gpsimd
            dmp.dma_start(rep(slot, 0, [[1024, 32], [1, 1024]]), o8s[:])
        # gathers (both mh at once): per v: dst [128, 128] s-block;
        # src addr = mh*32768 + (8Q + 2v + e)*1024 + hf*512 + n*128 + s
        slot0 = o8_scr[2 * j, :]
        for v in range(4):
            tgt = sig_all if v == 0 else rgb_all[v - 1]
            dst = tgt[:, S * j:S * (j + 1)]
            src = rep(slot0, 2 * v * 1024,
                      [[32768, 2], [8192, 4], [1, 2048]])
            nc.sync.dma_start(dst, src)

    # ================= compositing (j-half granularity, overlapped)
    # persistent across halves
    w_all = work.tile([128, J * S], dt.float32)
    sd = work.tile([128, J * S], dt.float32)
    nc.vector.memset(sd[:], 0.0)
    wsum = pr.tile([128, J], dt.float32)
    wtsum = pr.tile([128, J], dt.float32)

    def composite_sigma(j0, j1):
        c0, c1 = S * j0, S * j1
        jn = j1 - j0
        sig_e = work.tile([128, jn * S], dt.float32, tag="bigB", name=f"sig_e{j0}")
        SC.activation(sig_e[:], sig_all[:, c0:c1], AF.Exp)
        sig_e1 = work.tile([128, jn * S], dt.float32, tag="bigC", name=f"sig_e1{j0}")
        V.tensor_scalar(sig_e1[:], sig_e[:], 1.0, None, ALU.add)
        sig_sp = work.tile([128, jn * S], dt.float32, tag="bigA", name=f"sig_sp{j0}")
        SC.activation(sig_sp[:], sig_e1[:], AF.Ln)                 # softplus
        negds = work.tile([128, jn * S], dt.float32, tag="bigB", name=f"negds{j0}")
        for j in range(j0, j1):
            V.tensor_scalar(negds[:, S * (j - j0):S * (j - j0 + 1)],
                            sig_sp[:, S * (j - j0):S * (j - j0 + 1)],
                            negdelta[:, j:j + 1], None, ALU.mult)
        am1 = work.tile([128, jn * S], dt.float32, tag="bigC", name=f"am1{j0}")
        SC.activation(am1[:], negds[:], AF.Exp)                    # 1 - alpha
        sh = work.tile([128, jn * S], dt.float32, tag="bigA", name=f"sh{j0}")
        V.tensor_scalar(sh[:], am1[:], 1e-15, None, ALU.add)
        sd_v = sd[:, c0:c1].rearrange("p (j s) -> p j s", s=S)
        sh_v = sh[:].rearrange("p (j s) -> p j s", s=S)
        nc.gpsimd.tensor_copy(sd_v[:, :, 1:S], sh_v[:, :, 0:S - 1])
        tex = work.tile([128, jn * S], dt.float32, tag="bigB", name=f"tex{j0}")
        V.tensor_tensor_scan(tex[:], sd[:, c0:c1], d1c[:, c0:c1], 0.0,
                             ALU.mult, ALU.add)
        alpha = work.tile([128, jn * S], dt.float32, tag="bigA", name=f"alpha{j0}")
        V.tensor_scalar(alpha[:], am1[:], -1.0, 1.0, ALU.mult, ALU.add)
        V.tensor_tensor(w_all[:, c0:c1], alpha[:], tex[:], ALU.mult)
        nc.scalar.dma_start(w_out[:, c0:c1], w_all[:, c0:c1])
        w_v = w_all[:, c0:c1].rearrange("p (j s) -> p j s", s=S)
        V.tensor_reduce(wsum[:, j0:j1], w_v, mybir.AxisListType.X, ALU.add)
        wt = work.tile([128, jn * S], dt.float32, tag="bigC", name=f"wt{j0}")
        nc.gpsimd.tensor_tensor(wt[:], w_all[:, c0:c1], t_full[:, c0:c1], ALU.mult)
        V.tensor_reduce(wtsum[:, j0:j1], wt[:].rearrange("p (j s) -> p j s", s=S),
                        mybir.AxisListType.X, ALU.add)

    # image part, deferred so tanh needs one table load at the very end
    def composite_image():
        nc.scalar.dma_start(wsum_out[:], wsum[:])
        dep1 = pr.tile([128, J], dt.float32)
        V.tensor_tensor(dep1[:], near2[:], wsum[:], ALU.mult)
        dep2 = pr.tile([128, J], dt.float32)
        V.tensor_tensor(dep2[:], rangez[:], wtsum[:], ALU.mult)
        depth = pr.tile([128, J], dt.float32)
        V.tensor_tensor(depth[:], dep1[:], dep2[:], ALU.add)
        nc.scalar.dma_start(depth_out[:], depth[:])
        img = pr.tile([128, 3 * J], dt.float32)
        imgv = img[:].rearrange("p (j c) -> p j c", c=3)
        for c in range(3):
            th = work.tile([128, J * S], dt.float32, name=f"th{c}", tag="bigB")
            SC.activation(th[:], rgb_all[c][:], AF.Tanh, scale=0.5)
            wth = work.tile([128, J * S], dt.float32, name=f"wth{c}", tag="bigC")
            eng = nc.gpsimd if c != 2 else V
            eng.tensor_tensor(wth[:], w_all[:], th[:], ALU.mult)
            sc_ = pr.tile([128, J], dt.float32, name=f"sc{c}", tag="scc")
            V.tensor_reduce(sc_[:], wth[:].rearrange("p (j s) -> p j s", s=S),
                            mybir.AxisListType.X, ALU.add)
            tmp2 = pr.tile([128, J], dt.float32, name=f"tmp2{c}", tag="tmp2")
            V.tensor_tensor(tmp2[:], wsum[:], sc_[:], ALU.add)
            V.tensor_scalar(imgv[:, :, c:c + 1].squeeze(-1), tmp2[:], 0.5, None,
                            ALU.mult)
        nc.scalar.dma_start(img_out[:], img[:])

    for j in range(J):
        main_tile(j)
        if (j + 1) % 2 == 0 and j < J - 1:
            composite_sigma(j - 1, j + 1)
    composite_sigma(J - 2, J)
    composite_image()


# ======================================================================
# SPMD entry
# ======================================================================
_CACHE = {}


def _build_bass():
    nc = bacc.Bacc("TRN2", target_bir_lowering=False, debug=False,
                   num_devices=NCORES)
    ins_specs = [
        ("abi_in", [J, 16, 16 * 128], MMDT),
        ("near2_in", [128, J], dt.float32),
        ("rangez_in", [128, J], dt.float32),
        ("negdelta_in", [128, J], dt.float32),
        ("lhsT2_in", [128, 4 * 128], MMDT),
        ("RC_in", [16, 512], MMDT),
        ("t_full_in", [128, J * S], dt.float32),
        ("d1_in", [128, J * S], dt.float32),
    ]
    outs_specs = [
        ("w_out", [128, J * S]), ("depth_out", [128, J]),
        ("wsum_out", [128, J]), ("img_out", [128, 3 * J]),
    ]
    ins = [nc.dram_tensor(n, s, dty, kind="ExternalInput").ap()
           for n, s, dty in ins_specs]
    outs = [nc.dram_tensor(n, s, dt.float32, kind="ExternalOutput").ap()
            for n, s in outs_specs]
    with tile.TileContext(nc) as tc:
        with ExitStack() as ctx:
            nerf_kernel(ctx, tc, outs, ins)
    nc.compile()
    return nc


def get_nc():
    if "nc" not in _CACHE:
        _CACHE["nc"] = _build_bass()
    return _CACHE["nc"]


def make_in_maps(rays_o, rays_d, W1, W2, b1, w_sigma, W_color):
    consts = build_consts(W1, W2, b1, w_sigma, W_color)
    o_flat = np.ascontiguousarray(rays_o.reshape(-1, 3), f32)
    d_flat = np.ascontiguousarray(rays_d.reshape(-1, 3), f32)
    in_maps = []
    for c in range(NCORES):
        abi, near2, rangez, negdelta = per_ray_host(
            o_flat[c * R:(c + 1) * R], d_flat[c * R:(c + 1) * R],
            W1.astype(f32), W2.astype(f32), b1.astype(f32))
        in_maps.append({
            "abi_in": abi, "near2_in": near2, "rangez_in": rangez,
            "negdelta_in": negdelta, "lhsT2_in": consts["lhsT2"],
            "RC_in": consts["RC"], "t_full_in": consts["t_full"],
            "d1_in": consts["d1"],
        })
    return in_maps


def assemble_outputs(results, B, N):
    imgs, depths, ws, wsums = [], [], [], []
    for c in range(len(results)):
        r = results[c]
        imgs.append(unshuffle_img(r["img_out"]))
        depths.append(unshuffle_scalar(r["depth_out"]))
        ws.append(unshuffle_w(r["w_out"]))
        wsums.append(unshuffle_scalar(r["wsum_out"]))
    image = np.concatenate(imgs, 0).reshape(B, N, 3)
    depth = np.concatenate(depths, 0).reshape(B, N)
    weights = np.concatenate(ws, 0).reshape(B, N, S)
    weights_sum = np.concatenate(wsums, 0).reshape(B, N)
    return image, depth, weights, weights_sum


def run_spmd(inputs, trace=False):
    from concourse import bass_utils
    nc = get_nc()
    in_maps = make_in_maps(**inputs)[:NCORES_ENV]
    res = bass_utils.run_bass_kernel_spmd(
        nc, in_maps, core_ids=list(range(NCORES_ENV)), trace=trace)
    return res


def kernel(rays_o, rays_d, W1, W2, b1, w_sigma, W_color):
    inputs = dict(rays_o=np.asarray(rays_o, f32), rays_d=np.asarray(rays_d, f32),
                  W1=np.asarray(W1, f32), W2=np.asarray(W2, f32),
                  b1=np.asarray(b1, f32), w_sigma=np.asarray(w_sigma, f32),
                  W_color=np.asarray(W_color, f32))
    B, N = inputs["rays_o"].shape[:2]
    res = run_spmd(inputs)
    return assemble_outputs(res.results, B, N)


# revision 31
# speedup vs baseline: 1.0167x; 1.0022x over previous
"""NeRF renderer Bass kernel for TRN2 (8-core data-parallel over rays).

Restructured math (validated vs reference to ~5e-6 rel in numpy):
  pre[h, (n,s)] = A[ray, h] + t[s] * B[ray, h]        (A/B precomputed on host)
  h = relu(pre);  sigma = softplus(h @ w_sigma);  rgb = sigmoid(h @ W_color)
  weights via exact exclusive-cumprod (hardware tensor_tensor_scan).

Per-core geometry (2048 rays, 128 samples, hidden 64):
  ray-local index u = 128*j + p   (j = ray-tile 0..15, p = partition 0..127)
  within tile j:  p = 64*mh + 16*Q + 8*e + 4*hf + n
  group g = 8*mh + 2*Q + hf  (8 rays: even chunk e=0 n=0..3, odd chunk e=1)

Device pipeline per core:
  per ray-tile j: DMA block-diag ABi [16, 2048] (f32r);
    per O8-half mh: 4x { mm1 x2 (K=16, M=128, N=512, f32r) -> H2 psum
                         relu (ACT/DVE) -> h2s sbuf (f32r)
                         mm2 x2 (K=128, M=128 accum over quads) -> O8 psum }
    drain O8 -> sbuf -> dump valid rows to DRAM -> gather sigma/rgb
      into ray-major [128, J*S] tiles
  compositing: softplus (exp/ln), exp, tensor_tensor_scan cumprod,
    per-block reductions, sigmoid-via-tanh, dense output DMAs.
"""
import dataclasses
import os
import numpy as np
from contextlib import ExitStack

import concourse.bass as bass
import concourse.tile as tile
from concourse import bacc, mybir

f32 = np.float32
dt = mybir.dt
AF = mybir.ActivationFunctionType
ALU = mybir.AluOpType

NCORES = 8
R = 2048          # rays per core
J = 16            # ray tiles per core
S = 128           # samples per ray
H = 64            # hidden dim
ZCLAMP = 4.0
MIN_NEAR = 0.05
USE_F32R = True
MMDT = dt.float32r if USE_F32R else dt.float32
NCORES_ENV = int(os.environ.get("NERF_CORES", "8"))


def rep(ap, offset_delta, new_ap):
    return dataclasses.replace(ap, offset=ap.offset + offset_delta, ap=new_ap)


# ---------------------------------------------------------------- host side
def build_consts(W1, W2, b1, w_sigma, W_color):
    t = np.linspace(0.0, 1.0, S, dtype=f32)
    # mm2 weights: M=128, one [128,128] block per quad Q (values at cols 32Q+r)
    # col order r = 2*v + e  (v: 0=sigma,1=r,2=g,3=b ; e: chunk parity)
    lhsT2 = np.zeros((128, 4, 128), f32)
    wout = np.concatenate([w_sigma, W_color], 1).astype(f32)      # [64, 4]
    for Q in range(4):
        for v in range(4):
            lhsT2[0:64, Q, 8 * Q + 2 * v + 0] = wout[:, v]
            lhsT2[64:128, Q, 8 * Q + 2 * v + 1] = wout[:, v]
    lhsT2 = lhsT2.reshape(128, 4 * 128)
    # mm1 rhs constant [16, 512]: cols (n:4, s:128)
    RC = np.zeros((16, 512), f32)
    for n in range(4):
        RC[n, n * S:(n + 1) * S] = 1.0            # A-selector
        RC[4 + n, n * S:(n + 1) * S] = t          # B-selector * t
    RC[8:16] = RC[0:8]
    t_full = np.tile(t[None, :], (128, J)).astype(f32)            # [128, 2048]
    d1 = np.zeros((128, J * S), f32)
    d1[:, ::S] = 1.0                                              # scan reset
    return dict(lhsT2=lhsT2, RC=RC, t_full=t_full, d1=d1)


def per_ray_host(o, d, W1, W2, b1):
    """o, d: [R, 3] one core. Returns ABi [J, 16, 2048], near2/rangez/negdelta
    in the [128, J] shuffled layout."""
    inv = (f32(1.0) / (d + f32(1e-15))).astype(f32)
    tmin = ((f32(-1.0) - o) * inv).astype(f32)
    tmax = ((f32(1.0) - o) * inv).astype(f32)
    near = np.minimum(tmin, tmax).max(-1)
    far = np.maximum(tmin, tmax).min(-1)
    miss = far < near
    near = np.where(miss, f32(1e9), near).astype(f32)
    far = np.where(miss, f32(1e9), far).astype(f32)
    near = np.maximum(near, f32(MIN_NEAR))
    rangez = (far - near).astype(f32)
    negdelta = (rangez * f32(-1.0 / (S - 1))).astype(f32)
    nearc = np.minimum(near, f32(ZCLAMP))
    farc = np.minimum(far, f32(ZCLAMP))
    rangec = (farc - nearc).astype(f32)
    P = (o + d * nearc[:, None]).astype(f32)
    Qv = (d * rangec[:, None]).astype(f32)
    A = (P @ W1 + d @ W2 + b1).astype(f32)        # [R, 64]
    Bq = (Qv @ W1).astype(f32)                    # [R, 64]

    # block-diag ABi: [J, 16, 16, 128]: (j, k, g, m128)
    # ray u = 128*j + p ; p = 64mh + 16Q + 8e + 4hf + n ; g = 8mh + 2Q + hf
    # K-rows: k = 8e + n (A), 8e + 4 + n (B); cols 64e + (0..63)
    pvec = np.arange(128)
    mh, Qq, e, hf, n = (pvec // 64, (pvec // 16) % 4, (pvec // 8) % 2,
                        (pvec // 4) % 2, pvec % 4)
    gv = 8 * mh + 2 * Qq + hf
    kA = 8 * e + n
    kB = 8 * e + 4 + n
    col0 = 64 * e
    A3 = A.reshape(J, 128, H)
    B3 = Bq.reshape(J, 128, H)
    abi = np.zeros((J, 16, 16, 128), f32)
    mgrid = np.arange(64)
    abi[:, kA[:, None], gv[:, None], (col0[:, None] + mgrid[None, :])] = A3
    abi[:, kB[:, None], gv[:, None], (col0[:, None] + mgrid[None, :])] = B3
    abi = np.ascontiguousarray(abi.reshape(J, 16, 16 * 128))

    def shuf(x):
        return np.ascontiguousarray(x.reshape(J, 128).T)
    return abi, shuf(near), shuf(rangez), shuf(negdelta)


def unshuffle_w(w_out):
    return w_out.reshape(128, J, S).transpose(1, 0, 2).reshape(R, S)


def unshuffle_scalar(x_out):
    return x_out.reshape(128, J).T.reshape(R)


def unshuffle_img(img_out):
    return img_out.reshape(128, J, 3).transpose(1, 0, 2).reshape(R, 3)


# ---------------------------------------------------------------- kernel body
def nerf_kernel(ctx: ExitStack, tc: tile.TileContext, outs, ins):
    nc = tc.nc
    (abi_in, near2_in, rangez_in, negdelta_in,
     lhsT2_in, RC_in, t_full_in, d1_in) = ins
    (w_out, depth_out, wsum_out, img_out) = outs
    V = nc.vector
    SC = nc.scalar

    const = ctx.enter_context(tc.tile_pool(name="const", bufs=1))
    pr = ctx.enter_context(tc.tile_pool(name="pr", bufs=1))
    work = ctx.enter_context(tc.tile_pool(name="work", bufs=1))
    abipool = ctx.enter_context(tc.tile_pool(name="abip", bufs=4))
    hpool = ctx.enter_context(tc.tile_pool(name="hp", bufs=4))
    ospool = ctx.enter_context(tc.tile_pool(name="osp", bufs=4))
    psum = ctx.enter_context(tc.tile_pool(name="psum", bufs=1, space="PSUM"))
    dram = ctx.enter_context(tc.tile_pool(name="dram", bufs=1, space="DRAM"))

    def cload(shape, src, dtype=dt.float32):
        tl = const.tile(shape, dtype, name=f"c_{src.tensor.name}")
        eng = nc.sync if src.dtype == dtype else nc.gpsimd
        eng.dma_start(tl[:], src[:])
        return tl

    lhsT2 = cload([128, 4 * 128], lhsT2_in, MMDT)
    RC = cload([16, 512], RC_in, MMDT)
    t_full = cload([128, J * S], t_full_in)
    d1c = cload([128, J * S], d1_in)
    near2 = cload([128, J], near2_in)
    rangez = cload([128, J], rangez_in)
    negdelta = cload([128, J], negdelta_in)

    sig_all = work.tile([128, J * S], dt.float32)
    rgb_all = [work.tile([128, J * S], dt.float32, name=f"rgb_all{c}",
                         tag=f"rgb{c}") for c in range(3)]
    o8_scr = dram.tile([J * 2, 32 * 1024], dt.float32)

    # ================= main loop
    half_done = [False]

    def main_tile(j):
        abi = abipool.tile([16, 16 * 128], MMDT, name="abi", tag="abi")
        nc.gpsimd.dma_start(abi[:], abi_in[j, :, :])
        for mh in range(2):
            o8 = psum.tile([128, 1024], dt.float32, name="o8", tag="o8", bufs=2)
            for Q in range(4):
                h2 = psum.tile([128, 1024], dt.float32, name="h2", tag="h2", bufs=2)
                for hf in range(2):
                    g = 8 * mh + 2 * Q + hf
                    nc.tensor.matmul(h2[:, 512 * hf:512 * (hf + 1)],
                                     abi[:, 128 * g:128 * (g + 1)],
                                     RC[:], start=True, stop=True)
                h2s = hpool.tile([128, 1024], MMDT, name="h2s", tag="h2s")
                if (8 * j + 4 * mh + Q) % 8 in (2, 5, 7):
                    V.tensor_scalar(h2s[:], h2[:], 0.0, None, ALU.max)
                else:
                    SC.activation(h2s[:], h2[:], AF.Relu)
                for hf in range(2):
                    nc.tensor.matmul(o8[:, 512 * hf:512 * (hf + 1)],
                                     lhsT2[:, 128 * Q:128 * (Q + 1)],
                                     h2s[:, 512 * hf:512 * (hf + 1)],
                                     start=(Q == 0), stop=(Q == 3))
            # drain O8 valid rows 0:32 -> sbuf
            o8s = ospool.tile([32, 1024], dt.float32, name="o8s", tag="o8s")
            if (2 * j + mh) % 2 == 0:
                SC.copy(o8s[:], o8[0:32, :])
            else:
                V.tensor_copy(o8s[:], o8[0:32, :])
            # single dump [32, 1024] -> scratch slot (contiguous)
            slot = o8_scr[2 * j + mh, :]
            dmp = nc.sync if mh == 0 else nc.gpsimd
            dmp.dma_start(rep(slot, 0, [[1024, 32], [1, 1024]]), o8s[:])
        # gathers (both mh at once): per v: dst [128, 128] s-block;
        # src addr = mh*32768 + (8Q + 2v + e)*1024 + hf*512 + n*128 + s
        slot0 = o8_scr[2 * j, :]
        for v in range(4):
            tgt = sig_all if v == 0 else rgb_all[v - 1]
            dst = tgt[:, S * j:S * (j + 1)]
            src = rep(slot0, 2 * v * 1024,
                      [[32768, 2], [8192, 4], [1, 2048]])
            nc.sync.dma_start(dst, src)

    # ================= compositing (j-half granularity, overlapped)
    # persistent across halves
    w_all = work.tile([128, J * S], dt.float32)
    sd = work.tile([128, J * S], dt.float32)
    nc.vector.memset(sd[:], 0.0)
    wsum = pr.tile([128, J], dt.float32)
    wtsum = pr.tile([128, J], dt.float32)

    def composite_sigma(j0, j1):
        c0, c1 = S * j0, S * j1
        jn = j1 - j0
        sig_e = work.tile([128, jn * S], dt.float32, tag="bigB", name=f"sig_e{j0}")
        SC.activation(sig_e[:], sig_all[:, c0:c1], AF.Exp)
        sig_e1 = work.tile([128, jn * S], dt.float32, tag="bigC", name=f"sig_e1{j0}")
        V.tensor_scalar(sig_e1[:], sig_e[:], 1.0, None, ALU.add)
        sig_sp = work.tile([128, jn * S], dt.float32, tag="bigA", name=f"sig_sp{j0}")
        SC.activation(sig_sp[:], sig_e1[:], AF.Ln)                 # softplus
        negds = work.tile([128, jn * S], dt.float32, tag="bigB", name=f"negds{j0}")
        for j in range(j0, j1):
            V.tensor_scalar(negds[:, S * (j - j0):S * (j - j0 + 1)],
                            sig_sp[:, S * (j - j0):S * (j - j0 + 1)],
                            negdelta[:, j:j + 1], None, ALU.mult)
        am1 = work.tile([128, jn * S], dt.float32, tag="bigC", name=f"am1{j0}")
        SC.activation(am1[:], negds[:], AF.Exp)                    # 1 - alpha
        sh = work.tile([128, jn * S], dt.float32, tag="bigA", name=f"sh{j0}")
        V.tensor_scalar(sh[:], am1[:], 1e-15, None, ALU.add)
        sd_v = sd[:, c0:c1].rearrange("p (j s) -> p j s", s=S)
        sh_v = sh[:].rearrange("p (j s) -> p j s", s=S)
        nc.gpsimd.tensor_copy(sd_v[:, :, 1:S], sh_v[:, :, 0:S - 1])
        tex = work.tile([128, jn * S], dt.float32, tag="bigB", name=f"tex{j0}")
        V.tensor_tensor_scan(tex[:], sd[:, c0:c1], d1c[:, c0:c1], 0.0,
                             ALU.mult, ALU.add)
        alpha = work.tile([128, jn * S], dt.float32, tag="bigA", name=f"alpha{j0}")
        V.tensor_scalar(alpha[:], am1[:], -1.0, 1.0, ALU.mult, ALU.add)
        V.tensor_tensor(w_all[:, c0:c1], alpha[:], tex[:], ALU.mult)
        nc.scalar.dma_start(w_out[:, c0:c1], w_all[:, c0:c1])
        w_v = w_all[:, c0:c1].rearrange("p (j s) -> p j s", s=S)
        V.tensor_reduce(wsum[:, j0:j1], w_v, mybir.AxisListType.X, ALU.add)
        wt = work.tile([128, jn * S], dt.float32, tag="bigC", name=f"wt{j0}")
        nc.gpsimd.tensor_tensor(wt[:], w_all[:, c0:c1], t_full[:, c0:c1], ALU.mult)
        V.tensor_reduce(wtsum[:, j0:j1], wt[:].rearrange("p (j s) -> p j s", s=S),
                        mybir.AxisListType.X, ALU.add)

    # image part, deferred so tanh needs one table load at the very end
    def composite_image():
        nc.scalar.dma_start(wsum_out[:], wsum[:])
        dep1 = pr.tile([128, J], dt.float32)
        V.tensor_tensor(dep1[:], near2[:], wsum[:], ALU.mult)
        dep2 = pr.tile([128, J], dt.float32)
        V.tensor_tensor(dep2[:], rangez[:], wtsum[:], ALU.mult)
        depth = pr.tile([128, J], dt.float32)
        V.tensor_tensor(depth[:], dep1[:], dep2[:], ALU.add)
        nc.scalar.dma_start(depth_out[:], depth[:])
        img = pr.tile([128, 3 * J], dt.float32)
        imgv = img[:].rearrange("p (j c) -> p j c", c=3)
        for c in range(3):
            th = work.tile([128, J * S], dt.float32, name=f"th{c}", tag="bigB")
            SC.activation(th[:], rgb_all[c][:], AF.Tanh, scale=0.5)
            wth = work.tile([128, J * S], dt.float32, name=f"wth{c}", tag="bigC")
            eng = nc.gpsimd if c != 2 else V
            eng.tensor_tensor(wth[:], w_all[:], th[:], ALU.mult)
            sc_ = pr.tile([128, J], dt.float32, name=f"sc{c}", tag="scc")
            V.tensor_reduce(sc_[:], wth[:].rearrange("p (j s) -> p j s", s=S),
                            mybir.AxisListType.X, ALU.add)
            tmp2 = pr.tile([128, J], dt.float32, name=f"tmp2{c}", tag="tmp2")
            V.tensor_tensor(tmp2[:], wsum[:], sc_[:], ALU.add)
            V.tensor_scalar(imgv[:, :, c:c + 1].squeeze(-1), tmp2[:], 0.5, None,
                            ALU.mult)
        nc.scalar.dma_start(img_out[:], img[:])

    for j in range(J):
        main_tile(j)
        if (j + 1) % 2 == 0 and j < J - 1:
            composite_sigma(j - 1, j + 1)
        elif j == J - 1:
            composite_sigma(J - 2, J - 1)
    composite_sigma(J - 1, J)
    composite_image()


# ======================================================================
# SPMD entry
# ======================================================================
_CACHE = {}


def _build_bass():
    nc = bacc.Bacc("TRN2", target_bir_lowering=False, debug=False,
                   num_devices=NCORES)
    ins_specs = [
        ("abi_in", [J, 16, 16 * 128], MMDT),
        ("near2_in", [128, J], dt.float32),
        ("rangez_in", [128, J], dt.float32),
        ("negdelta_in", [128, J], dt.float32),
        ("lhsT2_in", [128, 4 * 128], MMDT),
        ("RC_in", [16, 512], MMDT),
        ("t_full_in", [128, J * S], dt.float32),
        ("d1_in", [128, J * S], dt.float32),
    ]
    outs_specs = [
        ("w_out", [128, J * S]), ("depth_out", [128, J]),
        ("wsum_out", [128, J]), ("img_out", [128, 3 * J]),
    ]
    ins = [nc.dram_tensor(n, s, dty, kind="ExternalInput").ap()
           for n, s, dty in ins_specs]
    outs = [nc.dram_tensor(n, s, dt.float32, kind="ExternalOutput").ap()
            for n, s in outs_specs]
    with tile.TileContext(nc) as tc:
        with ExitStack() as ctx:
            nerf_kernel(ctx, tc, outs, ins)
    nc.compile()
    return nc


def get_nc():
    if "nc" not in _CACHE:
        _CACHE["nc"] = _build_bass()
    return _CACHE["nc"]


def make_in_maps(rays_o, rays_d, W1, W2, b1, w_sigma, W_color):
    consts = build_consts(W1, W2, b1, w_sigma, W_color)
    o_flat = np.ascontiguousarray(rays_o.reshape(-1, 3), f32)
    d_flat = np.ascontiguousarray(rays_d.reshape(-1, 3), f32)
    in_maps = []
    for c in range(NCORES):
        abi, near2, rangez, negdelta = per_ray_host(
            o_flat[c * R:(c + 1) * R], d_flat[c * R:(c + 1) * R],
            W1.astype(f32), W2.astype(f32), b1.astype(f32))
        in_maps.append({
            "abi_in": abi, "near2_in": near2, "rangez_in": rangez,
            "negdelta_in": negdelta, "lhsT2_in": consts["lhsT2"],
            "RC_in": consts["RC"], "t_full_in": consts["t_full"],
            "d1_in": consts["d1"],
        })
    return in_maps


def assemble_outputs(results, B, N):
    imgs, depths, ws, wsums = [], [], [], []
    for c in range(len(results)):
        r = results[c]
        imgs.append(unshuffle_img(r["img_out"]))
        depths.append(unshuffle_scalar(r["depth_out"]))
        ws.append(unshuffle_w(r["w_out"]))
        wsums.append(unshuffle_scalar(r["wsum_out"]))
    image = np.concatenate(imgs, 0).reshape(B, N, 3)
    depth = np.concatenate(depths, 0).reshape(B, N)
    weights = np.concatenate(ws, 0).reshape(B, N, S)
    weights_sum = np.concatenate(wsums, 0).reshape(B, N)
    return image, depth, weights, weights_sum


def run_spmd(inputs, trace=False):
    from concourse import bass_utils
    nc = get_nc()
    in_maps = make_in_maps(**inputs)[:NCORES_ENV]
    res = bass_utils.run_bass_kernel_spmd(
        nc, in_maps, core_ids=list(range(NCORES_ENV)), trace=trace)
    return res


def kernel(rays_o, rays_d, W1, W2, b1, w_sigma, W_color):
    inputs = dict(rays_o=np.asarray(rays_o, f32), rays_d=np.asarray(rays_d, f32),
                  W1=np.asarray(W1, f32), W2=np.asarray(W2, f32),
                  b1=np.asarray(b1, f32), w_sigma=np.asarray(w_sigma, f32),
                  W_color=np.asarray(W_color, f32))
    B, N = inputs["rays_o"].shape[:2]
    res = run_spmd(inputs)
    return assemble_outputs(res.results, B, N)
